# revision 45
# baseline (speedup 1.0000x reference)
"""Trainium2 Bass kernel for batched 2D attention with relative position bias.

Reference computation (per batch image, C=512 channels, n=1024 positions):
    qkv = W @ x            # [3C, n] 1x1 conv
    S   = q^T k + pos^T q  # [n, n] logits
    A   = softmax(S, axis=-1)
    out = v @ A^T          # [C, n]

Distribution: pure data parallel over batch (64 images -> 8 NeuronCores x 8).
W, rel_h, rel_w replicated. No collectives.

Algebraic reductions:
  * S = x^T (Wq^T Wk) x + pos^T Wq x.  M = Wq^T Wk and PF = Wq^T [rel_h|rel_w]
    are precomputed on the host in float64: the device computes g = M x
    (one projection instead of q and k).
  * The [n,n] logits are computed TRANSPOSED (S^T tiles, partition = key
    index m): stationary g-blocks x moving x.  The softmaxed tile then IS
    the moving operand A^T of out = v A^T -- no PE transposes of A needed.
  * The rank-64 positional term exponentiates separably:
        exp(S - 90) = exp(S1 - 30) * E1[m, h(n)] * E2[m, w(n)]
    with E = exp(PF^T x - 30).  The E1*E2 broadcast products (stride-0
    free-dim APs) are prebuilt per m-tile on the otherwise idle GpSimd,
    replacing the 0/1-selector matmuls of the row formulation; the at-tile
    multiply is then a single full-rate packed bf16 DVE op.
  * Softmax denominators (column sums of the unnormalized A^T) come from a
    ones-stationary matmul over a bf16 running sum kept on DVE (emitted one
    tile ahead of at production so the in-order DVE queue never stalls the
    PE); the divide lands in SBUF via ~18-bit fast-approx DVE reciprocals,
    applied in place on GpSimd/DVE with each output half DMA'd as it lands.

Scheduling: everything for image b+1 that the PE needs at the image
boundary (x DMA, PF^T x matmuls + exp, the 8 E^T transposes as one
contiguous burst with two batched scalar PSUM evictions) is emitted from
the middle of image b's out phase, so the PE matmul stream crosses image
boundaries with zero semaphore waits and never drops out of the 2.4GHz
pstate (a stall >~0.1us drops the Tensor clock to 1.2GHz for up to 3us).

Engine assignment: PE does only matmuls (zero-gap stream); Scalar does the
exps and all PSUM->SBUF evictions (Copy/Exp share one activation table);
DVE does the at-multiplies, colsum chain and reciprocals; GpSimd does the
positional-factor products and half the output scalings.

Matmul precision: bfloat16 operands for the g/v/S projections+logits and
the output matmul -- not for FLOP rate (f32r and bf16 are both 1 cycle/row)
but for PACING: a bf16 stationary loads in 97ns and hides under the 213ns
matmul, while f32r's 187ns LDWEIGHTS + swap latch paced those phases at
~227ns/matmul (bf16: ~216, worth ~14us).  Accuracy: bf16 x/M/g add ~5e-3
logit noise which softmax renormalization largely cancels (measured l2
5.3e-3 vs the 2e-2 budget).  emit_t (PF^T x) stays f32r: bf16 there would
triple the positional-factor error.  x is loaded f32r once and cast to a
bf16 copy by four DVE tensor_copys in out-phase slack.  Output is stored
bf16 (halves the clustered store traffic and the end-of-kernel drain) and
cast back to f32 on the host.

Measured dead ends (do not retry): fp8 DoubleRow anywhere (needs hi+lo
splits that cost as much as one bf16 pass at equal accuracy); a PE dummy
-matmul pstate pre-ramp during the DMA-bound head (a zero-idle run trips
the sustained-power governor and caps the WHOLE run at ~2.0GHz, -85us);
issuing warmup weight DMAs from the Scalar engine's DGE (its init work
delays the issues, -8us).
"""

import sys

if "/opt/trn_rl_repo" not in sys.path:
    sys.path.insert(0, "/opt/trn_rl_repo")

import numpy as np

import concourse.bass as bass
import concourse.tile as tile
from concourse import bacc, mybir
from concourse.bass_utils import run_bass_kernel_spmd
from concourse.masks import make_identity

F32 = mybir.dt.float32
F32R = mybir.dt.float32r
BF16 = mybir.dt.bfloat16
F16 = mybir.dt.float16

B, C, H, W_ = 64, 512, 32, 32
N = H * W_              # 1024 positions
NCORES = 8
BLOC = B // NCORES      # 8 images per core
CT = C // 128           # 4 channel tiles
NT = N // 128           # 8 position tiles
P = 128
JW = 64                 # rel-pos rank: 32 (h) + 32 (w)
EXP_BIAS = -30.0        # 3 * 30 = 90 total shift, matches |S| < ~85 bound


def _round_f32r(a):
    """Round float32 -> float32r (11-bit mantissa) exactly as the hardware
    cast does, returning a float32-typed array with rounded bits."""
    from neuronxcc.starfish.support.dtype import static_cast_fp32_to_fp32r
    return np.asarray(static_cast_fp32_to_fp32r(
        np.ascontiguousarray(a, dtype=np.float32))).view(np.float32)


def build_nc():
    nc = bacc.Bacc("TRN2", target_bir_lowering=False, debug=False,
                   num_devices=NCORES)
    x_ext = nc.declare_dram_parameter("x", [BLOC, C, N], F32R, isOutput=False)
    # M and Wv ship as bf16: the g/v/S matmuls run with bf16 operands, whose
    # 97ns LDWEIGHTS hides under the 213ns matmul (f32r's 187ns load + swap
    # latch paced those phases at ~227ns/matmul instead of ~216).
    mt_ext = nc.declare_dram_parameter("MT", [C, C], F16, isOutput=False)
    wvt_ext = nc.declare_dram_parameter("WVT", [C, C], F16, isOutput=False)
    pf_ext = nc.declare_dram_parameter("PF", [C, JW], F32R, isOutput=False)
    # bf16 output: halves store traffic (the DMA engines sustain only
    # ~100-200GB/s for the clustered out-phase stores) and halves the
    # end-of-kernel drain; the host casts back to f32.  Quantization adds
    # ~1.1e-3 RMS to a 2.0e-3 baseline -- far inside the 2e-2 budget.
    o_ext = nc.declare_dram_parameter("out", [BLOC, C, N], BF16, isOutput=True)

    import contextlib
    with tile.TileContext(nc) as tc, contextlib.ExitStack() as _stk:
        # ExitStack keeps the pool count out of Python's 20-nested-block
        # compile limit (each manager in a multi-`with` counts as a block).
        _p = lambda *a, **k: _stk.enter_context(tc.tile_pool(*a, **k))
        if True:
            const = _p(name="const", bufs=1)
            wtp = _p(name="wt", bufs=1)
            xfp = _p(name="xf", bufs=2)
            xbfp = _p(name="xbf", bufs=2)
            gp = _p(name="gp", bufs=2)
            vtp = _p(name="vt", bufs=2)
            Ep = _p(name="Ep", bufs=2)
            ep = _p(name="ep", bufs=2)
            e12p = _p(name="e12", bufs=NT - 1)
            atp = _p(name="at", bufs=1)
            accp = _p(name="acc", bufs=2)
            rrp = _p(name="rr", bufs=1)
            osbp = _p(name="osb", bufs=4)
            obfp = _p(name="obf", bufs=3)
            pbig = _p(name="pbig", bufs=2, space="PSUM")
            pvp = _p(name="pv", bufs=2, space="PSUM")
            ptt = _p(name="ptt", bufs=2, space="PSUM")
            ident = const.tile([P, P], BF16, tag="id")
            make_identity(nc, ident[:])
            nbias = const.tile([P, 1], F32, tag="nbias")
            nc.vector.memset(nbias[:], EXP_BIAS)
            ones_b = const.tile([P, P], BF16, tag="ones_b")
            nc.vector.memset(ones_b[:], 1.0)

            # (No PE pre-ramp burst here: filling the PE's idle DMA-wait
            # head with dummy matmuls to pre-ramp the 2.4GHz pstate was
            # measured to backfire -- with zero idle anywhere the
            # sustained-power governor caps the whole run at ~2.0GHz,
            # costing ~85us against the ~3us the warm clock saves.)

            # one-time weights (host-precomputed, f32r-rounded).  Few BIG
            # DMA issues instead of 20 small ones: the Sync engine can only
            # issue a DMA every ~600ns, so the 20-issue warmup serialization
            # (not HBM bandwidth) was pacing the first matmuls.  pf + xf0
            # chunks lead (emit_t's inputs), mtw interleaves (g's input),
            # wvt trails (v phase is ~25us in).
            mtw = wtp.tile([P, CT, C], F16, tag="mtw")
            wvt = wtp.tile([P, CT, C], F16, tag="wvt")
            pf = wtp.tile([P, CT, JW], F32R, tag="pf")
            xf0 = xfp.tile([P, CT, N], F32R, tag="xf")
            # (all warmup issues stay on Sync: issuing the weight DMAs from
            # the Scalar engine's DGE "in parallel" was measured ~8us
            # SLOWER -- scalar's init/table-load work delays its issues)
            nc.sync.dma_start(
                pf[:], pf_ext.rearrange("(ct p) j -> p ct j", p=P))
            nc.sync.dma_start(xf0[:, 0], x_ext[0, 0:P, :])
            nc.sync.dma_start(xf0[:, 1], x_ext[0, P:2 * P, :])
            nc.sync.dma_start(
                mtw[:, 0:2],
                mt_ext[0:2 * P, :].rearrange("(ct p) c -> p ct c", p=P))
            nc.sync.dma_start(xf0[:, 2], x_ext[0, 2 * P:3 * P, :])
            nc.sync.dma_start(xf0[:, 3], x_ext[0, 3 * P:4 * P, :])
            nc.sync.dma_start(
                mtw[:, 2:4],
                mt_ext[2 * P:4 * P, :].rearrange("(ct p) c -> p ct c", p=P))
            nc.sync.dma_start(
                wvt[:], wvt_ext.rearrange("(ct p) c -> p ct c", p=P))

            # bf16 copy of x for the g/v/S matmuls (emit_t keeps the f32r
            # original: bf16 there would triple the positional-factor
            # error).  Four DVE casts run in DVE slack right after the
            # reciprocals; per-chunk so warmup casts chase the x DMAs.
            def emit_xbf(xf_in):
                xbf_ = xbfp.tile([P, CT, N], F16, tag="xbf")
                for cc in range(CT):
                    nc.vector.tensor_copy(xbf_[:, cc], xf_in[:, cc])
                return xbf_

            # t = PF^T x : [j 0:64, m] (rows 0:32 rel_h^T q, 32:64
            # rel_w^T q), followed by E = exp(t - 30) bf16 [64, m].  Emitted
            # for image b+1 from the MIDDLE of image b's out phase (software
            # pipelining across images): every PE instruction crossing the
            # image boundary then has its semaphores satisfied well in
            # advance, so the PE stream never breaks and the pstate clock
            # ramp is preserved.
            def emit_t(xf_in):
                pst = pbig.tile([P, N], F32, tag="pbig")
                for kt in range(CT):
                    for nb in range(2):
                        nc.tensor.matmul(
                            pst[0:JW, nb * 512:(nb + 1) * 512],
                            pf[:, kt],
                            xf_in[:, kt, nb * 512:(nb + 1) * 512],
                            start=(kt == 0), stop=(kt == CT - 1),
                        )
                E = Ep.tile([P, N], BF16, tag="E")
                nc.scalar.activation(E[0:JW, :], pst[0:JW, :],
                                     mybir.ActivationFunctionType.Exp,
                                     bias=nbias[0:JW], scale=1.0)
                return E

            # E^T tiles [m-part, j] via one contiguous burst of PE
            # transposes, 4 per PSUM tile, evicted by TWO scalar copies
            # instead of 8 DVE copies.  Emitted for image b+1 from image b's
            # out phase (after ct2), ~2.5us after emit_t's exp: the
            # transposes' inputs are long since ready, so they never clog
            # the PE wait queue and never interleave (with stationary-reload
            # penalties) into the next image's matmul stream.
            def emit_trans(E_in):
                esb_ = ep.tile([P, NT, JW], BF16, tag="e")
                for g4 in range(2):
                    pE = ptt.tile([P, 4, JW], BF16, tag="ptt")
                    for k in range(4):
                        mt = g4 * 4 + k
                        nc.tensor.transpose(
                            pE[:, k],
                            E_in[0:JW, mt * P:(mt + 1) * P],
                            ident[0:JW, 0:JW],
                        )
                    nc.scalar.activation(
                        esb_[:, g4 * 4:(g4 + 1) * 4, :].rearrange(
                            "p a b -> p (a b)"),
                        pE[:].rearrange("p a b -> p (a b)"),
                        mybir.ActivationFunctionType.Copy,
                        bias=0.0, scale=1.0)
                return esb_

            xf = xf0
            xbf = emit_xbf(xf0)
            Esb = emit_t(xf0)

            # ---- per image ----
            for b in range(BLOC):

                # g = (Wq^T Wk) x : [c-part, oi, n] bf16
                g = gp.tile([P, CT, N], F16, tag="g")
                for oi in range(CT):
                    psg = pbig.tile([P, N], F32, tag="pbig")
                    for kt in range(CT):
                        for nb in range(2):
                            nc.tensor.matmul(
                                psg[:, nb * 512:(nb + 1) * 512],
                                mtw[:, kt, oi * P:(oi + 1) * P],
                                xbf[:, kt, nb * 512:(nb + 1) * 512],
                                start=(kt == 0), stop=(kt == CT - 1),
                            )
                    nc.scalar.activation(g[:, oi], psg[:],
                                         mybir.ActivationFunctionType.Copy,
                                         bias=0.0, scale=1.0)

                # image 0's E^T burst sits after the g phase (the scalar exp
                # of Esb has completed by then); images 1.. get esb from the
                # previous image's out phase.
                if b == 0:
                    esb = emit_trans(Esb)

                # v^T : [m-part, mt, c] bf16
                vt = vtp.tile([P, NT, C], BF16, tag="vt")
                for mt in range(NT):
                    psv = pvp.tile([P, 512], F32, tag="pv")
                    for kt in range(CT):
                        nc.tensor.matmul(
                            psv[:],
                            xbf[:, kt, mt * P:(mt + 1) * P],
                            wvt[:, kt],
                            start=(kt == 0), stop=(kt == CT - 1),
                        )
                    nc.scalar.activation(vt[:, mt], psv[:],
                                         mybir.ActivationFunctionType.Copy,
                                         bias=0.0, scale=1.0)


                # separable positional factor products, built ahead on GpSimd
                # (depend only on esb): e12[mt][m, n] = E1[m, h(n)] * E2[m, w(n)]
                e12s = []
                for mt in range(NT):
                    e1 = esb[:, mt, 0:H].unsqueeze(2).broadcast_to([P, H, W_])
                    e2 = esb[:, mt, H:JW].unsqueeze(1).broadcast_to([P, H, W_])
                    e12 = e12p.tile([P, N], BF16, tag="e12")
                    nc.gpsimd.tensor_tensor(
                        e12[:].rearrange("p (h w) -> p h w", h=H),
                        e1, e2, mybir.AluOpType.mult)
                    e12s.append(e12)

                # attention columns: S^T tiles -> unnormalized A^T (bf16)
                at = atp.tile([P, NT, N], BF16, tag="at")
                acc = accp.tile([P, N], BF16, tag="acc")
                for mt in range(NT):
                    psT = pbig.tile([P, N], F32, tag="pbig")
                    for ci in range(CT):
                        for nb in range(2):
                            nc.tensor.matmul(
                                psT[:, nb * 512:(nb + 1) * 512],
                                g[:, ci, mt * P:(mt + 1) * P],
                                xbf[:, ci, nb * 512:(nb + 1) * 512],
                                start=(ci == 0), stop=(ci == CT - 1),
                            )
                    nc.scalar.activation(at[:, mt], psT[:],
                                         mybir.ActivationFunctionType.Exp,
                                         bias=nbias[:], scale=1.0)
                    # column-sum accumulation on DVE in bf16 (2x DVE rate,
                    # ~0.6us/add; the ~0.2% denominator rounding is well
                    # inside the error budget), emitted one tile behind the
                    # at-mults BEFORE tile mt's own mults so acc is complete
                    # ~1.2us earlier and the colsum matmuls mid-out-ct0
                    # never wait on the DVE; tile 7 folds into the colsum
                    # matmul.
                    if mt == 1:
                        nc.vector.tensor_copy(acc[:], at[:, 0])
                    elif mt > 1:
                        nc.vector.tensor_tensor(acc[:], acc[:], at[:, mt - 1],
                                                mybir.AluOpType.add)
                    # at[m, n] *= e12[m, n]  (plain packed bf16 mult, in
                    # place, full tile: one DVE op per tile keeps the
                    # near-saturated S-phase DVE chain (add + mult per
                    # 1.71us of PE work) under the PE pace)
                    nc.vector.tensor_tensor(at[:, mt], at[:, mt],
                                            e12s[mt][:],
                                            mybir.AluOpType.mult)

                # out = v A^T : [c-part, n], normalized by column sums.
                # The colsum matmuls (into pv-pool banks, idle after the v
                # phase) are emitted mid-way through out-ct0 so the DVE
                # reciprocal halves can overlap the remaining out matmuls;
                # each reciprocal half is interleaved between psO copies so
                # it never delays a pbig bank release.
                rrec = rrp.tile([P, N], F32, tag="rrec")
                obs = []
                pcss = []

                def norm_store(ct_):
                    # normalize in halves on GpSimd and DVE in parallel,
                    # casting to the bf16 store tile, DMAing each half as it
                    # completes.  Streamed from inside the out loop (obs[0]
                    # after ct2, obs[1] after ct3, rest behind the loop) so
                    # the stores overlap the remaining out matmuls and the
                    # end-of-kernel drain is ~2 half-size tiles instead of 4.
                    obf = obfp.tile([P, N], BF16, tag="obf")
                    for hb in range(2):
                        eng = nc.gpsimd if hb == 0 else nc.vector
                        sl = slice(hb * 512, (hb + 1) * 512)
                        eng.tensor_tensor(obf[:, sl], obs[ct_][:, sl],
                                          rrec[:, sl], mybir.AluOpType.mult)
                        nc.sync.dma_start(
                            o_ext[b, ct_ * P:(ct_ + 1) * P,
                                  hb * 512:(hb + 1) * 512],
                            obf[:, sl])

                for ct in range(CT):
                    if b == BLOC - 1 and ct >= 2:
                        # kernel-tail critical path: nb-major matmul order
                        # with a SEPARATE pvp PSUM tile per 512-col half
                        # (the pv banks idle after colsum/recips; separate
                        # tiles dodge the tile-granular WAR that serialized
                        # nb1's matmuls on nb0's eviction when both halves
                        # shared one psO tile).  Each half is normalized
                        # fused into the DVE eviction and stored the moment
                        # its 8-matmul sweep lands, alternating the issue
                        # between two DGE engines -- only the final 128KB
                        # half remains for the end-of-kernel drain.
                        if ct == 2:
                            norm_store(0)
                            norm_store(1)
                        obf = obfp.tile([P, N], BF16, tag="obf")
                        for nb in range(2):
                            sl = slice(nb * 512, (nb + 1) * 512)
                            psH = pvp.tile([P, 512], F32, tag="pv")
                            for mt in range(NT):
                                nc.tensor.matmul(
                                    psH[:],
                                    vt[:, mt, ct * P:(ct + 1) * P],
                                    at[:, mt, sl],
                                    start=(mt == 0), stop=(mt == NT - 1),
                                )
                            nc.vector.tensor_tensor(
                                obf[:, sl], psH[:], rrec[:, sl],
                                mybir.AluOpType.mult)
                            eng = nc.scalar if nb == 0 else nc.sync
                            eng.dma_start(
                                o_ext[b, ct * P:(ct + 1) * P, sl],
                                obf[:, sl])
                        continue
                    psO = pbig.tile([P, N], F32, tag="pbig")
                    for mt in range(NT):
                        for nb in range(2):
                            nc.tensor.matmul(
                                psO[:, nb * 512:(nb + 1) * 512],
                                vt[:, mt, ct * P:(ct + 1) * P],
                                at[:, mt, nb * 512:(nb + 1) * 512],
                                start=(mt == 0), stop=(mt == NT - 1),
                            )
                    if ct == 0:
                        # colsum broadcast, emitted AFTER ct0's matmuls:
                        # psCS[p, n] = sum_i acc[i, n] + sum_i at7[i, n].
                        # The at7/acc DVE chain finishes ~psT7+2us; at the
                        # post-loop emission point the PE arrives ~psT7+3.4us
                        # so neither the colsum nor out-mt7 ever waits.
                        for nb in range(2):
                            pcs = pvp.tile([P, 512], F32, tag="pv")
                            nc.tensor.matmul(
                                pcs[:],
                                ones_b[:],
                                acc[:, nb * 512:(nb + 1) * 512],
                                start=True, stop=False,
                            )
                            nc.tensor.matmul(
                                pcs[:],
                                ones_b[:],
                                at[:, NT - 1, nb * 512:(nb + 1) * 512],
                                start=False, stop=True,
                            )
                            pcss.append(pcs)
                    ob = osbp.tile([P, N], F32, tag="osb")
                    # psO -> SBUF copies on the Scalar engine (Copy shares
                    # the Exp activation table, so no table reload), keeping
                    # PSUM-bank releases off the DVE queue.
                    nc.scalar.activation(ob[:], psO[:],
                                         mybir.ActivationFunctionType.Copy,
                                         bias=0.0, scale=1.0)
                    if ct < 2:
                        # 18-bit ~0.7us approx reciprocal (vs 3.3us exact):
                        # colsums are strictly-positive well-normalized f32,
                        # far from the approx's undefined edge cases, and the
                        # denominator error budget is ~1e-3.
                        nc.vector.reciprocal_approx_fast(
                            rrec[:, ct * 512:(ct + 1) * 512], pcss[ct][:])
                    obs.append(ob)
                    if ct == 1 and b + 1 < BLOC:
                        xf_next = xfp.tile([P, CT, N], F32R, tag="xf")
                        nc.sync.dma_start(
                            xf_next[:],
                            x_ext[b + 1].rearrange("(ct p) n -> p ct n", p=P))
                        Esb_next = emit_t(xf_next)
                        xbf_next = emit_xbf(xf_next)
                    if ct == 2:
                        norm_store(0)
                    if ct == 3:
                        norm_store(1)
                        norm_store(2)
                if b + 1 < BLOC:
                    # E^T burst for image b+1 sits between ct3's bf16
                    # matmuls and the f32r g phase: its two stationary-mode
                    # switches merge into the dtype seam that exists at the
                    # image boundary anyway, instead of splitting ct2/ct3.
                    esb_next = emit_trans(Esb_next)
                    norm_store(3)
                    xf, xbf, Esb, esb = xf_next, xbf_next, Esb_next, esb_next

    nc.compile()
    return nc


_NC_CACHE = None


def _get_nc():
    global _NC_CACHE
    if _NC_CACHE is None:
        _NC_CACHE = build_nc()
    return _NC_CACHE


def _prep_inputs(x, W, rel_h, rel_w):
    x = np.ascontiguousarray(np.asarray(x, dtype=np.float32))
    W = np.asarray(W, dtype=np.float32).astype(np.float64)
    rel_hm = np.asarray(rel_h, dtype=np.float32).reshape(C, H).astype(np.float64)
    rel_wm = np.asarray(rel_w, dtype=np.float32).reshape(C, W_).astype(np.float64)
    Wq, Wk, Wv = W[0:C], W[C:2 * C], W[2 * C:3 * C]
    # S = q^T k + pos^T q = x^T (Wq^T Wk) x + (Wq^T pos)^T x, with the rank-64
    # pos term separable into h- and w-factors applied post-exponentiation.
    mt_h = np.ascontiguousarray(
        (Wq.T @ Wk).T.astype(np.float32)).astype(np.float16)
    wvt_h = np.ascontiguousarray(
        Wv.T.astype(np.float32)).astype(np.float16)
    pfm = np.zeros((C, JW), np.float64)
    pfm[:, 0:H] = Wq.T @ rel_hm
    pfm[:, H:JW] = Wq.T @ rel_wm
    pf_h = _round_f32r(pfm)
    xs = _round_f32r(x).reshape(NCORES, BLOC, C, N)
    return xs, mt_h, wvt_h, pf_h


def _in_maps(inputs):
    xs, mt_h, wvt_h, pf_h = _prep_inputs(**inputs)
    return [
        {"x": np.ascontiguousarray(xs[i]), "MT": mt_h, "WVT": wvt_h,
         "PF": pf_h}
        for i in range(NCORES)
    ]


def kernel(x, W, rel_h, rel_w):
    nc = _get_nc()
    in_maps = _in_maps({"x": x, "W": W, "rel_h": rel_h, "rel_w": rel_w})
    res = run_bass_kernel_spmd(nc, in_maps, core_ids=list(range(NCORES)))
    out = np.concatenate(
        [np.asarray(res.results[i]["out"]).astype(np.float32)
         for i in range(NCORES)], axis=0)
    return out.reshape(B, C, H, W_)



# revision 47
# speedup vs baseline: 1.0052x; 1.0052x over previous
"""Trainium2 Bass kernel for batched 2D attention with relative position bias.

Reference computation (per batch image, C=512 channels, n=1024 positions):
    qkv = W @ x            # [3C, n] 1x1 conv
    S   = q^T k + pos^T q  # [n, n] logits
    A   = softmax(S, axis=-1)
    out = v @ A^T          # [C, n]

Distribution: pure data parallel over batch (64 images -> 8 NeuronCores x 8).
W, rel_h, rel_w replicated. No collectives.

Algebraic reductions:
  * S = x^T (Wq^T Wk) x + pos^T Wq x.  M = Wq^T Wk and PF = Wq^T [rel_h|rel_w]
    are precomputed on the host in float64: the device computes g = M x
    (one projection instead of q and k).
  * The [n,n] logits are computed TRANSPOSED (S^T tiles, partition = key
    index m): stationary g-blocks x moving x.  The softmaxed tile then IS
    the moving operand A^T of out = v A^T -- no PE transposes of A needed.
  * The rank-64 positional term exponentiates separably:
        exp(S - 90) = exp(S1 - 30) * E1[m, h(n)] * E2[m, w(n)]
    with E = exp(PF^T x - 30).  The E1*E2 broadcast products (stride-0
    free-dim APs) are prebuilt per m-tile on the otherwise idle GpSimd,
    replacing the 0/1-selector matmuls of the row formulation; the at-tile
    multiply is then a single full-rate packed bf16 DVE op.
  * Softmax denominators (column sums of the unnormalized A^T) come from a
    ones-stationary matmul over a bf16 running sum kept on DVE (emitted one
    tile ahead of at production so the in-order DVE queue never stalls the
    PE); the divide lands in SBUF via ~18-bit fast-approx DVE reciprocals,
    applied in place on GpSimd/DVE with each output half DMA'd as it lands.

Scheduling: everything for image b+1 that the PE needs at the image
boundary (x DMA, PF^T x matmuls + exp, the 8 E^T transposes as one
contiguous burst with two batched scalar PSUM evictions) is emitted from
the middle of image b's out phase, so the PE matmul stream crosses image
boundaries with zero semaphore waits and never drops out of the 2.4GHz
pstate (a stall >~0.1us drops the Tensor clock to 1.2GHz for up to 3us).

Engine assignment: PE does only matmuls (zero-gap stream); Scalar does the
exps and all PSUM->SBUF evictions (Copy/Exp share one activation table);
DVE does the at-multiplies, colsum chain and reciprocals; GpSimd does the
positional-factor products and half the output scalings.

Matmul precision: FLOAT16 operands for x/M/Wv/g in the g/v/S
projections+logits -- not for FLOP rate (f32r and 16-bit are all 1
cycle/row) but for PACING: a 16-bit stationary loads in 97ns and hides
under the 213ns matmul, while f32r's 187ns LDWEIGHTS + swap latch paced
those phases at ~227ns/matmul (16-bit: ~216, worth ~14us).  fp16 (10
mantissa bits) over bf16 (7): same speed, 8x less logit noise (measured
l2 2.6e-3 / maxrel 6.7e-3 vs bf16's 5.3e-3 / 1.9e-2, budget 2e-2); all
pre-exponential tensors fit fp16's range, and sub-normal x entries
(|x|<6e-5) flushing is negligible.  The post-exp side (at/E/e12/vt, the
out matmul, output store) stays bf16 -- those values span e^-60..1 and
NEED bf16's 8 exponent bits.  emit_t (PF^T x) stays f32r: 16-bit there
triples the positional-factor error.  x is loaded f32r once and cast to
an fp16 copy by four DVE tensor_copys in out-phase slack.  Output is
stored bf16 (halves the clustered store traffic and end-of-kernel drain)
and cast back to f32 on the host.

Measured dead ends (do not retry): fp8 DoubleRow anywhere (needs hi+lo
splits that cost as much as one bf16 pass at equal accuracy); a PE dummy
-matmul pstate pre-ramp during the DMA-bound head (a zero-idle run trips
the sustained-power governor and caps the WHOLE run at ~2.0GHz, -85us);
issuing warmup weight DMAs from the Scalar engine's DGE (its init work
delays the issues, -8us).
"""

import sys

if "/opt/trn_rl_repo" not in sys.path:
    sys.path.insert(0, "/opt/trn_rl_repo")

import numpy as np

import concourse.bass as bass
import concourse.tile as tile
from concourse import bacc, mybir
from concourse.bass_utils import run_bass_kernel_spmd
from concourse.masks import make_identity

F32 = mybir.dt.float32
F32R = mybir.dt.float32r
BF16 = mybir.dt.bfloat16
F16 = mybir.dt.float16

B, C, H, W_ = 64, 512, 32, 32
N = H * W_              # 1024 positions
NCORES = 8
BLOC = B // NCORES      # 8 images per core
CT = C // 128           # 4 channel tiles
NT = N // 128           # 8 position tiles
P = 128
JW = 64                 # rel-pos rank: 32 (h) + 32 (w)
EXP_BIAS = -30.0        # 3 * 30 = 90 total shift, matches |S| < ~85 bound


def _round_f32r(a):
    """Round float32 -> float32r (11-bit mantissa) exactly as the hardware
    cast does, returning a float32-typed array with rounded bits."""
    from neuronxcc.starfish.support.dtype import static_cast_fp32_to_fp32r
    return np.asarray(static_cast_fp32_to_fp32r(
        np.ascontiguousarray(a, dtype=np.float32))).view(np.float32)


def build_nc():
    nc = bacc.Bacc("TRN2", target_bir_lowering=False, debug=False,
                   num_devices=NCORES)
    x_ext = nc.declare_dram_parameter("x", [BLOC, C, N], F16, isOutput=False)
    # M and Wv ship as fp16: the g/v/S matmuls run with 16-bit operands,
    # whose 97ns LDWEIGHTS hides under the 213ns matmul (f32r's 187ns load +
    # swap latch paced those phases at ~227ns/matmul instead of ~216); fp16
    # over bf16 for 8x the mantissa at identical speed.
    mt_ext = nc.declare_dram_parameter("MT", [C, C], F16, isOutput=False)
    wvt_ext = nc.declare_dram_parameter("WVT", [C, C], F16, isOutput=False)
    pf_ext = nc.declare_dram_parameter("PF", [C, JW], F16, isOutput=False)
    # bf16 output: halves store traffic (the DMA engines sustain only
    # ~100-200GB/s for the clustered out-phase stores) and halves the
    # end-of-kernel drain; the host casts back to f32.  Quantization adds
    # ~1.1e-3 RMS to a 2.0e-3 baseline -- far inside the 2e-2 budget.
    o_ext = nc.declare_dram_parameter("out", [BLOC, C, N], BF16, isOutput=True)

    import contextlib
    with tile.TileContext(nc) as tc, contextlib.ExitStack() as _stk:
        # ExitStack keeps the pool count out of Python's 20-nested-block
        # compile limit (each manager in a multi-`with` counts as a block).
        _p = lambda *a, **k: _stk.enter_context(tc.tile_pool(*a, **k))
        if True:
            const = _p(name="const", bufs=1)
            wtp = _p(name="wt", bufs=1)
            xbfp = _p(name="xbf", bufs=2)
            gp = _p(name="gp", bufs=2)
            vtp = _p(name="vt", bufs=2)
            Ep = _p(name="Ep", bufs=2)
            ep = _p(name="ep", bufs=2)
            e12p = _p(name="e12", bufs=NT - 1)
            atp = _p(name="at", bufs=1)
            accp = _p(name="acc", bufs=2)
            rrp = _p(name="rr", bufs=1)
            osbp = _p(name="osb", bufs=4)
            obfp = _p(name="obf", bufs=3)
            pbig = _p(name="pbig", bufs=2, space="PSUM")
            pvp = _p(name="pv", bufs=2, space="PSUM")
            ptt = _p(name="ptt", bufs=2, space="PSUM")
            ident = const.tile([P, P], BF16, tag="id")
            make_identity(nc, ident[:])
            nbias = const.tile([P, 1], F32, tag="nbias")
            nc.vector.memset(nbias[:], EXP_BIAS)
            ones_b = const.tile([P, P], BF16, tag="ones_b")
            nc.vector.memset(ones_b[:], 1.0)

            # (No PE pre-ramp burst here: filling the PE's idle DMA-wait
            # head with dummy matmuls to pre-ramp the 2.4GHz pstate was
            # measured to backfire -- with zero idle anywhere the
            # sustained-power governor caps the whole run at ~2.0GHz,
            # costing ~85us against the ~3us the warm clock saves.)

            # one-time weights (host-precomputed, f32r-rounded).  Few BIG
            # DMA issues instead of 20 small ones: the Sync engine can only
            # issue a DMA every ~600ns, so the 20-issue warmup serialization
            # (not HBM bandwidth) was pacing the first matmuls.  pf + xf0
            # chunks lead (emit_t's inputs), mtw interleaves (g's input),
            # wvt trails (v phase is ~25us in).
            mtw = wtp.tile([P, CT, C], F16, tag="mtw")
            wvt = wtp.tile([P, CT, C], F16, tag="wvt")
            pf = wtp.tile([P, CT, JW], F16, tag="pf")
            xf0 = xbfp.tile([P, CT, N], F16, tag="xbf")
            # (all warmup issues stay on Sync: issuing the weight DMAs from
            # the Scalar engine's DGE "in parallel" was measured ~8us
            # SLOWER -- scalar's init/table-load work delays its issues)
            nc.sync.dma_start(
                pf[:], pf_ext.rearrange("(ct p) j -> p ct j", p=P))
            nc.sync.dma_start(xf0[:, 0], x_ext[0, 0:P, :])
            nc.sync.dma_start(xf0[:, 1], x_ext[0, P:2 * P, :])
            nc.sync.dma_start(
                mtw[:, 0:2],
                mt_ext[0:2 * P, :].rearrange("(ct p) c -> p ct c", p=P))
            nc.sync.dma_start(xf0[:, 2], x_ext[0, 2 * P:3 * P, :])
            nc.sync.dma_start(xf0[:, 3], x_ext[0, 3 * P:4 * P, :])
            nc.sync.dma_start(
                mtw[:, 2:4],
                mt_ext[2 * P:4 * P, :].rearrange("(ct p) c -> p ct c", p=P))
            nc.sync.dma_start(
                wvt[:], wvt_ext.rearrange("(ct p) c -> p ct c", p=P))

            # t = PF^T x : [j 0:64, m] (rows 0:32 rel_h^T q, 32:64
            # rel_w^T q), followed by E = exp(t - 30) bf16 [64, m].  Emitted
            # for image b+1 from the MIDDLE of image b's out phase (software
            # pipelining across images): every PE instruction crossing the
            # image boundary then has its semaphores satisfied well in
            # advance, so the PE stream never breaks and the pstate clock
            # ramp is preserved.
            def emit_t(xf_in):
                pst = pbig.tile([P, N], F32, tag="pbig")
                for kt in range(CT):
                    for nb in range(2):
                        nc.tensor.matmul(
                            pst[0:JW, nb * 512:(nb + 1) * 512],
                            pf[:, kt],
                            xf_in[:, kt, nb * 512:(nb + 1) * 512],
                            start=(kt == 0), stop=(kt == CT - 1),
                        )
                E = Ep.tile([P, N], BF16, tag="E")
                nc.scalar.activation(E[0:JW, :], pst[0:JW, :],
                                     mybir.ActivationFunctionType.Exp,
                                     bias=nbias[0:JW], scale=1.0)
                return E

            # E^T tiles [m-part, j] via one contiguous burst of PE
            # transposes, 4 per PSUM tile, evicted by TWO scalar copies
            # instead of 8 DVE copies.  Emitted for image b+1 from image b's
            # out phase (after ct2), ~2.5us after emit_t's exp: the
            # transposes' inputs are long since ready, so they never clog
            # the PE wait queue and never interleave (with stationary-reload
            # penalties) into the next image's matmul stream.
            def emit_trans(E_in):
                esb_ = ep.tile([P, NT, JW], BF16, tag="e")
                for g4 in range(2):
                    pE = ptt.tile([P, 4, JW], BF16, tag="ptt")
                    for k in range(4):
                        mt = g4 * 4 + k
                        nc.tensor.transpose(
                            pE[:, k],
                            E_in[0:JW, mt * P:(mt + 1) * P],
                            ident[0:JW, 0:JW],
                        )
                    nc.scalar.activation(
                        esb_[:, g4 * 4:(g4 + 1) * 4, :].rearrange(
                            "p a b -> p (a b)"),
                        pE[:].rearrange("p a b -> p (a b)"),
                        mybir.ActivationFunctionType.Copy,
                        bias=0.0, scale=1.0)
                return esb_

            xbf = xf0
            Esb = emit_t(xf0)

            # ---- per image ----
            for b in range(BLOC):

                # g = (Wq^T Wk) x : [c-part, oi, n] bf16
                g = gp.tile([P, CT, N], F16, tag="g")
                for oi in range(CT):
                    psg = pbig.tile([P, N], F32, tag="pbig")
                    for kt in range(CT):
                        for nb in range(2):
                            nc.tensor.matmul(
                                psg[:, nb * 512:(nb + 1) * 512],
                                mtw[:, kt, oi * P:(oi + 1) * P],
                                xbf[:, kt, nb * 512:(nb + 1) * 512],
                                start=(kt == 0), stop=(kt == CT - 1),
                            )
                    nc.scalar.activation(g[:, oi], psg[:],
                                         mybir.ActivationFunctionType.Copy,
                                         bias=0.0, scale=1.0)

                # image 0's E^T burst sits after the g phase (the scalar exp
                # of Esb has completed by then); images 1.. get esb from the
                # previous image's out phase.
                if b == 0:
                    esb = emit_trans(Esb)

                # v^T : [m-part, mt, c] bf16
                vt = vtp.tile([P, NT, C], BF16, tag="vt")
                for mt in range(NT):
                    psv = pvp.tile([P, 512], F32, tag="pv")
                    for kt in range(CT):
                        nc.tensor.matmul(
                            psv[:],
                            xbf[:, kt, mt * P:(mt + 1) * P],
                            wvt[:, kt],
                            start=(kt == 0), stop=(kt == CT - 1),
                        )
                    nc.scalar.activation(vt[:, mt], psv[:],
                                         mybir.ActivationFunctionType.Copy,
                                         bias=0.0, scale=1.0)


                # separable positional factor products, built ahead on GpSimd
                # (depend only on esb): e12[mt][m, n] = E1[m, h(n)] * E2[m, w(n)]
                e12s = []
                for mt in range(NT):
                    e1 = esb[:, mt, 0:H].unsqueeze(2).broadcast_to([P, H, W_])
                    e2 = esb[:, mt, H:JW].unsqueeze(1).broadcast_to([P, H, W_])
                    e12 = e12p.tile([P, N], BF16, tag="e12")
                    nc.gpsimd.tensor_tensor(
                        e12[:].rearrange("p (h w) -> p h w", h=H),
                        e1, e2, mybir.AluOpType.mult)
                    e12s.append(e12)

                # attention columns: S^T tiles -> unnormalized A^T (bf16)
                at = atp.tile([P, NT, N], BF16, tag="at")
                acc = accp.tile([P, N], BF16, tag="acc")
                for mt in range(NT):
                    psT = pbig.tile([P, N], F32, tag="pbig")
                    for ci in range(CT):
                        for nb in range(2):
                            nc.tensor.matmul(
                                psT[:, nb * 512:(nb + 1) * 512],
                                g[:, ci, mt * P:(mt + 1) * P],
                                xbf[:, ci, nb * 512:(nb + 1) * 512],
                                start=(ci == 0), stop=(ci == CT - 1),
                            )
                    nc.scalar.activation(at[:, mt], psT[:],
                                         mybir.ActivationFunctionType.Exp,
                                         bias=nbias[:], scale=1.0)
                    # column-sum accumulation on DVE in bf16 (2x DVE rate,
                    # ~0.6us/add; the ~0.2% denominator rounding is well
                    # inside the error budget), emitted one tile behind the
                    # at-mults BEFORE tile mt's own mults so acc is complete
                    # ~1.2us earlier and the colsum matmuls mid-out-ct0
                    # never wait on the DVE; tile 7 folds into the colsum
                    # matmul.
                    if mt == 1:
                        nc.vector.tensor_copy(acc[:], at[:, 0])
                    elif mt > 1:
                        nc.vector.tensor_tensor(acc[:], acc[:], at[:, mt - 1],
                                                mybir.AluOpType.add)
                    # at[m, n] *= e12[m, n]  (plain packed bf16 mult, in
                    # place, full tile: one DVE op per tile keeps the
                    # near-saturated S-phase DVE chain (add + mult per
                    # 1.71us of PE work) under the PE pace)
                    nc.vector.tensor_tensor(at[:, mt], at[:, mt],
                                            e12s[mt][:],
                                            mybir.AluOpType.mult)

                # out = v A^T : [c-part, n], normalized by column sums.
                # The colsum matmuls (into pv-pool banks, idle after the v
                # phase) are emitted mid-way through out-ct0 so the DVE
                # reciprocal halves can overlap the remaining out matmuls;
                # each reciprocal half is interleaved between psO copies so
                # it never delays a pbig bank release.
                rrec = rrp.tile([P, N], F32, tag="rrec")
                obs = []
                pcss = []

                def norm_store(ct_):
                    # normalize in halves on GpSimd and DVE in parallel,
                    # casting to the bf16 store tile, DMAing each half as it
                    # completes.  Streamed from inside the out loop (obs[0]
                    # after ct2, obs[1] after ct3, rest behind the loop) so
                    # the stores overlap the remaining out matmuls and the
                    # end-of-kernel drain is ~2 half-size tiles instead of 4.
                    obf = obfp.tile([P, N], BF16, tag="obf")
                    for hb in range(2):
                        eng = nc.gpsimd if hb == 0 else nc.vector
                        sl = slice(hb * 512, (hb + 1) * 512)
                        eng.tensor_tensor(obf[:, sl], obs[ct_][:, sl],
                                          rrec[:, sl], mybir.AluOpType.mult)
                        nc.sync.dma_start(
                            o_ext[b, ct_ * P:(ct_ + 1) * P,
                                  hb * 512:(hb + 1) * 512],
                            obf[:, sl])

                for ct in range(CT):
                    if b == BLOC - 1 and ct >= 2:
                        # kernel-tail critical path: nb-major matmul order
                        # with a SEPARATE pvp PSUM tile per 512-col half
                        # (the pv banks idle after colsum/recips; separate
                        # tiles dodge the tile-granular WAR that serialized
                        # nb1's matmuls on nb0's eviction when both halves
                        # shared one psO tile).  Each half is normalized
                        # fused into the DVE eviction and stored the moment
                        # its 8-matmul sweep lands, alternating the issue
                        # between two DGE engines -- only the final 128KB
                        # half remains for the end-of-kernel drain.
                        if ct == 2:
                            norm_store(0)
                            norm_store(1)
                        obf = obfp.tile([P, N], BF16, tag="obf")
                        for nb in range(2):
                            sl = slice(nb * 512, (nb + 1) * 512)
                            psH = pvp.tile([P, 512], F32, tag="pv")
                            for mt in range(NT):
                                nc.tensor.matmul(
                                    psH[:],
                                    vt[:, mt, ct * P:(ct + 1) * P],
                                    at[:, mt, sl],
                                    start=(mt == 0), stop=(mt == NT - 1),
                                )
                            nc.vector.tensor_tensor(
                                obf[:, sl], psH[:], rrec[:, sl],
                                mybir.AluOpType.mult)
                            eng = nc.scalar if nb == 0 else nc.sync
                            eng.dma_start(
                                o_ext[b, ct * P:(ct + 1) * P, sl],
                                obf[:, sl])
                        continue
                    psO = pbig.tile([P, N], F32, tag="pbig")
                    for mt in range(NT):
                        for nb in range(2):
                            nc.tensor.matmul(
                                psO[:, nb * 512:(nb + 1) * 512],
                                vt[:, mt, ct * P:(ct + 1) * P],
                                at[:, mt, nb * 512:(nb + 1) * 512],
                                start=(mt == 0), stop=(mt == NT - 1),
                            )
                    if ct == 0:
                        # colsum broadcast, emitted AFTER ct0's matmuls:
                        # psCS[p, n] = sum_i acc[i, n] + sum_i at7[i, n].
                        # The at7/acc DVE chain finishes ~psT7+2us; at the
                        # post-loop emission point the PE arrives ~psT7+3.4us
                        # so neither the colsum nor out-mt7 ever waits.
                        for nb in range(2):
                            pcs = pvp.tile([P, 512], F32, tag="pv")
                            nc.tensor.matmul(
                                pcs[:],
                                ones_b[:],
                                acc[:, nb * 512:(nb + 1) * 512],
                                start=True, stop=False,
                            )
                            nc.tensor.matmul(
                                pcs[:],
                                ones_b[:],
                                at[:, NT - 1, nb * 512:(nb + 1) * 512],
                                start=False, stop=True,
                            )
                            pcss.append(pcs)
                    ob = osbp.tile([P, N], F32, tag="osb")
                    # psO -> SBUF copies on the Scalar engine (Copy shares
                    # the Exp activation table, so no table reload), keeping
                    # PSUM-bank releases off the DVE queue.
                    nc.scalar.activation(ob[:], psO[:],
                                         mybir.ActivationFunctionType.Copy,
                                         bias=0.0, scale=1.0)
                    if ct < 2:
                        # 18-bit ~0.7us approx reciprocal (vs 3.3us exact):
                        # colsums are strictly-positive well-normalized f32,
                        # far from the approx's undefined edge cases, and the
                        # denominator error budget is ~1e-3.
                        nc.vector.reciprocal_approx_fast(
                            rrec[:, ct * 512:(ct + 1) * 512], pcss[ct][:])
                    obs.append(ob)
                    if ct == 1 and b + 1 < BLOC:
                        xbf_next = xbfp.tile([P, CT, N], F16, tag="xbf")
                        nc.sync.dma_start(
                            xbf_next[:],
                            x_ext[b + 1].rearrange("(ct p) n -> p ct n", p=P))
                        Esb_next = emit_t(xbf_next)
                    if ct == 2:
                        norm_store(0)
                    if ct == 3:
                        norm_store(1)
                        norm_store(2)
                if b + 1 < BLOC:
                    # E^T burst for image b+1 sits between ct3's bf16
                    # matmuls and the f32r g phase: its two stationary-mode
                    # switches merge into the dtype seam that exists at the
                    # image boundary anyway, instead of splitting ct2/ct3.
                    esb_next = emit_trans(Esb_next)
                    norm_store(3)
                    xbf, Esb, esb = xbf_next, Esb_next, esb_next

    nc.compile()
    return nc


_NC_CACHE = None


def _get_nc():
    global _NC_CACHE
    if _NC_CACHE is None:
        _NC_CACHE = build_nc()
    return _NC_CACHE


def _prep_inputs(x, W, rel_h, rel_w):
    x = np.ascontiguousarray(np.asarray(x, dtype=np.float32))
    W = np.asarray(W, dtype=np.float32).astype(np.float64)
    rel_hm = np.asarray(rel_h, dtype=np.float32).reshape(C, H).astype(np.float64)
    rel_wm = np.asarray(rel_w, dtype=np.float32).reshape(C, W_).astype(np.float64)
    Wq, Wk, Wv = W[0:C], W[C:2 * C], W[2 * C:3 * C]
    # S = q^T k + pos^T q = x^T (Wq^T Wk) x + (Wq^T pos)^T x, with the rank-64
    # pos term separable into h- and w-factors applied post-exponentiation.
    mt_h = np.ascontiguousarray(
        (Wq.T @ Wk).T.astype(np.float32)).astype(np.float16)
    wvt_h = np.ascontiguousarray(
        Wv.T.astype(np.float32)).astype(np.float16)
    pfm = np.zeros((C, JW), np.float64)
    pfm[:, 0:H] = Wq.T @ rel_hm
    pfm[:, H:JW] = Wq.T @ rel_wm
    pf_h = pfm.astype(np.float32).astype(np.float16)
    xs = x.astype(np.float16).reshape(NCORES, BLOC, C, N)
    return xs, mt_h, wvt_h, pf_h


def _in_maps(inputs):
    xs, mt_h, wvt_h, pf_h = _prep_inputs(**inputs)
    return [
        {"x": np.ascontiguousarray(xs[i]), "MT": mt_h, "WVT": wvt_h,
         "PF": pf_h}
        for i in range(NCORES)
    ]


def kernel(x, W, rel_h, rel_w):
    nc = _get_nc()
    in_maps = _in_maps({"x": x, "W": W, "rel_h": rel_h, "rel_w": rel_w})
    res = run_bass_kernel_spmd(nc, in_maps, core_ids=list(range(NCORES)))
    out = np.concatenate(
        [np.asarray(res.results[i]["out"]).astype(np.float32)
         for i in range(NCORES)], axis=0)
    return out.reshape(B, C, H, W_)



# revision 48
# speedup vs baseline: 1.0057x; 1.0005x over previous
"""Trainium2 Bass kernel for batched 2D attention with relative position bias.

Reference computation (per batch image, C=512 channels, n=1024 positions):
    qkv = W @ x            # [3C, n] 1x1 conv
    S   = q^T k + pos^T q  # [n, n] logits
    A   = softmax(S, axis=-1)
    out = v @ A^T          # [C, n]

Distribution: pure data parallel over batch (64 images -> 8 NeuronCores x 8).
W, rel_h, rel_w replicated. No collectives.

Algebraic reductions:
  * S = x^T (Wq^T Wk) x + pos^T Wq x.  M = Wq^T Wk and PF = Wq^T [rel_h|rel_w]
    are precomputed on the host in float64: the device computes g = M x
    (one projection instead of q and k).
  * The [n,n] logits are computed TRANSPOSED (S^T tiles, partition = key
    index m): stationary g-blocks x moving x.  The softmaxed tile then IS
    the moving operand A^T of out = v A^T -- no PE transposes of A needed.
  * The rank-64 positional term exponentiates separably:
        exp(S - 90) = exp(S1 - 30) * E1[m, h(n)] * E2[m, w(n)]
    with E = exp(PF^T x - 30).  The E1*E2 broadcast products (stride-0
    free-dim APs) are prebuilt per m-tile on the otherwise idle GpSimd,
    replacing the 0/1-selector matmuls of the row formulation; the at-tile
    multiply is then a single full-rate packed bf16 DVE op.
  * Softmax denominators (column sums of the unnormalized A^T) come from a
    ones-stationary matmul over a bf16 running sum kept on DVE (emitted one
    tile ahead of at production so the in-order DVE queue never stalls the
    PE); the divide lands in SBUF via ~18-bit fast-approx DVE reciprocals,
    applied in place on GpSimd/DVE with each output half DMA'd as it lands.

Scheduling: everything for image b+1 that the PE needs at the image
boundary (x DMA, PF^T x matmuls + exp, the 8 E^T transposes as one
contiguous burst with two batched scalar PSUM evictions) is emitted from
the middle of image b's out phase, so the PE matmul stream crosses image
boundaries with zero semaphore waits and never drops out of the 2.4GHz
pstate (a stall >~0.1us drops the Tensor clock to 1.2GHz for up to 3us).

Engine assignment: PE does only matmuls (zero-gap stream); Scalar does the
exps and all PSUM->SBUF evictions (Copy/Exp share one activation table);
DVE does the at-multiplies, colsum chain and reciprocals; GpSimd does the
positional-factor products and half the output scalings.

Matmul precision: FLOAT16 operands for x/M/Wv/g in the g/v/S
projections+logits -- not for FLOP rate (f32r and 16-bit are all 1
cycle/row) but for PACING: a 16-bit stationary loads in 97ns and hides
under the 213ns matmul, while f32r's 187ns LDWEIGHTS + swap latch paced
those phases at ~227ns/matmul (16-bit: ~216, worth ~14us).  fp16 (10
mantissa bits) over bf16 (7): same speed, 8x less logit noise (measured
l2 2.7e-3 / maxrel 6.7e-3 vs bf16's 5.3e-3 / 1.9e-2, budget 2e-2); all
pre-exponential tensors fit fp16's range, and sub-normal x entries
(|x|<6e-5) flushing is negligible.  The post-exp side (at/E/e12/vt, the
out matmul, output store) stays bf16 -- those values span e^-60..1 and
NEED bf16's 8 exponent bits.  x and PF also ship as fp16 (emit_t's
positional factors only lose ~0.4% each at 10 mantissa bits -- measured
maxrel unchanged), which halves the x DMA traffic, removes the on-device
cast chain, and puts every matmul on 16-bit pacing.  Output is stored
bf16 (halves the clustered store traffic and end-of-kernel drain) and
cast back to f32 on the host.

Measured dead ends (do not retry): fp8 DoubleRow anywhere (needs hi+lo
splits that cost as much as one bf16 pass at equal accuracy); a PE dummy
-matmul pstate pre-ramp during the DMA-bound head (a zero-idle run trips
the sustained-power governor and caps the WHOLE run at ~2.0GHz, -85us);
issuing warmup weight DMAs from the Scalar engine's DGE (its init work
delays the issues, -8us).
"""

import sys

if "/opt/trn_rl_repo" not in sys.path:
    sys.path.insert(0, "/opt/trn_rl_repo")

import numpy as np

import concourse.bass as bass
import concourse.tile as tile
from concourse import bacc, mybir
from concourse.bass_utils import run_bass_kernel_spmd
from concourse.masks import make_identity

F32 = mybir.dt.float32
F32R = mybir.dt.float32r
BF16 = mybir.dt.bfloat16
F16 = mybir.dt.float16

B, C, H, W_ = 64, 512, 32, 32
N = H * W_              # 1024 positions
NCORES = 8
BLOC = B // NCORES      # 8 images per core
CT = C // 128           # 4 channel tiles
NT = N // 128           # 8 position tiles
P = 128
JW = 64                 # rel-pos rank: 32 (h) + 32 (w)
EXP_BIAS = -30.0        # 3 * 30 = 90 total shift, matches |S| < ~85 bound


def _round_f32r(a):
    """Round float32 -> float32r (11-bit mantissa) exactly as the hardware
    cast does, returning a float32-typed array with rounded bits."""
    from neuronxcc.starfish.support.dtype import static_cast_fp32_to_fp32r
    return np.asarray(static_cast_fp32_to_fp32r(
        np.ascontiguousarray(a, dtype=np.float32))).view(np.float32)


def build_nc():
    nc = bacc.Bacc("TRN2", target_bir_lowering=False, debug=False,
                   num_devices=NCORES)
    x_ext = nc.declare_dram_parameter("x", [BLOC, C, N], F16, isOutput=False)
    # M and Wv ship as fp16: the g/v/S matmuls run with 16-bit operands,
    # whose 97ns LDWEIGHTS hides under the 213ns matmul (f32r's 187ns load +
    # swap latch paced those phases at ~227ns/matmul instead of ~216); fp16
    # over bf16 for 8x the mantissa at identical speed.
    mt_ext = nc.declare_dram_parameter("MT", [C, C], F16, isOutput=False)
    wvt_ext = nc.declare_dram_parameter("WVT", [C, C], F16, isOutput=False)
    pf_ext = nc.declare_dram_parameter("PF", [C, JW], F16, isOutput=False)
    # bf16 output: halves store traffic (the DMA engines sustain only
    # ~100-200GB/s for the clustered out-phase stores) and halves the
    # end-of-kernel drain; the host casts back to f32.  Quantization adds
    # ~1.1e-3 RMS to a 2.0e-3 baseline -- far inside the 2e-2 budget.
    o_ext = nc.declare_dram_parameter("out", [BLOC, C, N], BF16, isOutput=True)

    import contextlib
    with tile.TileContext(nc) as tc, contextlib.ExitStack() as _stk:
        # ExitStack keeps the pool count out of Python's 20-nested-block
        # compile limit (each manager in a multi-`with` counts as a block).
        _p = lambda *a, **k: _stk.enter_context(tc.tile_pool(*a, **k))
        if True:
            const = _p(name="const", bufs=1)
            wtp = _p(name="wt", bufs=1)
            xbfp = _p(name="xbf", bufs=2)
            gp = _p(name="gp", bufs=2)
            vtp = _p(name="vt", bufs=2)
            Ep = _p(name="Ep", bufs=2)
            ep = _p(name="ep", bufs=2)
            e12p = _p(name="e12", bufs=NT - 1)
            atp = _p(name="at", bufs=1)
            accp = _p(name="acc", bufs=2)
            rrp = _p(name="rr", bufs=1)
            osbp = _p(name="osb", bufs=4)
            obfp = _p(name="obf", bufs=3)
            pbig = _p(name="pbig", bufs=2, space="PSUM")
            pvp = _p(name="pv", bufs=2, space="PSUM")
            ptt = _p(name="ptt", bufs=2, space="PSUM")
            ident = const.tile([P, P], BF16, tag="id")
            make_identity(nc, ident[:])
            nbias = const.tile([P, 1], F32, tag="nbias")
            nc.vector.memset(nbias[:], EXP_BIAS)
            ones_b = const.tile([P, P], BF16, tag="ones_b")
            nc.vector.memset(ones_b[:], 1.0)

            # (No PE pre-ramp burst here: filling the PE's idle DMA-wait
            # head with dummy matmuls to pre-ramp the 2.4GHz pstate was
            # measured to backfire -- with zero idle anywhere the
            # sustained-power governor caps the whole run at ~2.0GHz,
            # costing ~85us against the ~3us the warm clock saves.)

            # one-time weights (host-precomputed, f32r-rounded).  Few BIG
            # DMA issues instead of 20 small ones: the Sync engine can only
            # issue a DMA every ~600ns, so the 20-issue warmup serialization
            # (not HBM bandwidth) was pacing the first matmuls.  pf + xf0
            # chunks lead (emit_t's inputs), mtw interleaves (g's input),
            # wvt trails (v phase is ~25us in).
            mtw = wtp.tile([P, CT, C], F16, tag="mtw")
            wvt = wtp.tile([P, CT, C], F16, tag="wvt")
            pf = wtp.tile([P, CT, JW], F16, tag="pf")
            xf0 = xbfp.tile([P, CT, N], F16, tag="xbf")
            # (all warmup issues stay on Sync: issuing the weight DMAs from
            # the Scalar engine's DGE "in parallel" was measured ~8us
            # SLOWER -- scalar's init/table-load work delays its issues)
            nc.sync.dma_start(
                pf[:], pf_ext.rearrange("(ct p) j -> p ct j", p=P))
            nc.sync.dma_start(xf0[:, 0], x_ext[0, 0:P, :])
            nc.sync.dma_start(xf0[:, 1], x_ext[0, P:2 * P, :])
            nc.sync.dma_start(
                mtw[:, 0:2],
                mt_ext[0:2 * P, :].rearrange("(ct p) c -> p ct c", p=P))
            nc.sync.dma_start(xf0[:, 2], x_ext[0, 2 * P:3 * P, :])
            nc.sync.dma_start(xf0[:, 3], x_ext[0, 3 * P:4 * P, :])
            nc.sync.dma_start(
                mtw[:, 2:4],
                mt_ext[2 * P:4 * P, :].rearrange("(ct p) c -> p ct c", p=P))
            nc.sync.dma_start(
                wvt[:], wvt_ext.rearrange("(ct p) c -> p ct c", p=P))

            # t = PF^T x : [j 0:64, m] (rows 0:32 rel_h^T q, 32:64
            # rel_w^T q), followed by E = exp(t - 30) bf16 [64, m].  Emitted
            # for image b+1 from the MIDDLE of image b's out phase (software
            # pipelining across images): every PE instruction crossing the
            # image boundary then has its semaphores satisfied well in
            # advance, so the PE stream never breaks and the pstate clock
            # ramp is preserved.
            def emit_t(xf_in):
                pst = pbig.tile([P, N], F32, tag="pbig")
                for kt in range(CT):
                    for nb in range(2):
                        nc.tensor.matmul(
                            pst[0:JW, nb * 512:(nb + 1) * 512],
                            pf[:, kt],
                            xf_in[:, kt, nb * 512:(nb + 1) * 512],
                            start=(kt == 0), stop=(kt == CT - 1),
                        )
                E = Ep.tile([P, N], BF16, tag="E")
                nc.scalar.activation(E[0:JW, :], pst[0:JW, :],
                                     mybir.ActivationFunctionType.Exp,
                                     bias=nbias[0:JW], scale=1.0)
                return E

            # E^T tiles [m-part, j] via one contiguous burst of PE
            # transposes, 4 per PSUM tile, evicted by TWO scalar copies
            # instead of 8 DVE copies.  Emitted for image b+1 from image b's
            # out phase (after ct2), ~2.5us after emit_t's exp: the
            # transposes' inputs are long since ready, so they never clog
            # the PE wait queue and never interleave (with stationary-reload
            # penalties) into the next image's matmul stream.
            def emit_trans(E_in):
                esb_ = ep.tile([P, NT, JW], BF16, tag="e")
                for g4 in range(2):
                    pE = ptt.tile([P, 4, JW], BF16, tag="ptt")
                    for k in range(4):
                        mt = g4 * 4 + k
                        nc.tensor.transpose(
                            pE[:, k],
                            E_in[0:JW, mt * P:(mt + 1) * P],
                            ident[0:JW, 0:JW],
                        )
                    nc.scalar.activation(
                        esb_[:, g4 * 4:(g4 + 1) * 4, :].rearrange(
                            "p a b -> p (a b)"),
                        pE[:].rearrange("p a b -> p (a b)"),
                        mybir.ActivationFunctionType.Copy,
                        bias=0.0, scale=1.0)
                return esb_

            xbf = xf0
            Esb = emit_t(xf0)

            # ---- per image ----
            for b in range(BLOC):

                # g = (Wq^T Wk) x : [c-part, oi, n] bf16
                g = gp.tile([P, CT, N], F16, tag="g")
                for oi in range(CT):
                    psg = pbig.tile([P, N], F32, tag="pbig")
                    for kt in range(CT):
                        for nb in range(2):
                            nc.tensor.matmul(
                                psg[:, nb * 512:(nb + 1) * 512],
                                mtw[:, kt, oi * P:(oi + 1) * P],
                                xbf[:, kt, nb * 512:(nb + 1) * 512],
                                start=(kt == 0), stop=(kt == CT - 1),
                            )
                    nc.scalar.activation(g[:, oi], psg[:],
                                         mybir.ActivationFunctionType.Copy,
                                         bias=0.0, scale=1.0)

                # image 0's E^T burst sits after the g phase (the scalar exp
                # of Esb has completed by then); images 1.. get esb from the
                # previous image's out phase.
                if b == 0:
                    esb = emit_trans(Esb)

                # v^T : [m-part, mt, c] bf16
                vt = vtp.tile([P, NT, C], BF16, tag="vt")
                for mt in range(NT):
                    psv = pvp.tile([P, 512], F32, tag="pv")
                    for kt in range(CT):
                        nc.tensor.matmul(
                            psv[:],
                            xbf[:, kt, mt * P:(mt + 1) * P],
                            wvt[:, kt],
                            start=(kt == 0), stop=(kt == CT - 1),
                        )
                    nc.scalar.activation(vt[:, mt], psv[:],
                                         mybir.ActivationFunctionType.Copy,
                                         bias=0.0, scale=1.0)


                # separable positional factor products, built ahead on GpSimd
                # (depend only on esb): e12[mt][m, n] = E1[m, h(n)] * E2[m, w(n)]
                e12s = []
                for mt in range(NT):
                    e1 = esb[:, mt, 0:H].unsqueeze(2).broadcast_to([P, H, W_])
                    e2 = esb[:, mt, H:JW].unsqueeze(1).broadcast_to([P, H, W_])
                    e12 = e12p.tile([P, N], BF16, tag="e12")
                    nc.gpsimd.tensor_tensor(
                        e12[:].rearrange("p (h w) -> p h w", h=H),
                        e1, e2, mybir.AluOpType.mult)
                    e12s.append(e12)

                # attention columns: S^T tiles -> unnormalized A^T (bf16)
                at = atp.tile([P, NT, N], BF16, tag="at")
                acc = accp.tile([P, N], BF16, tag="acc")
                for mt in range(NT):
                    psT = pbig.tile([P, N], F32, tag="pbig")
                    for ci in range(CT):
                        for nb in range(2):
                            nc.tensor.matmul(
                                psT[:, nb * 512:(nb + 1) * 512],
                                g[:, ci, mt * P:(mt + 1) * P],
                                xbf[:, ci, nb * 512:(nb + 1) * 512],
                                start=(ci == 0), stop=(ci == CT - 1),
                            )
                    nc.scalar.activation(at[:, mt], psT[:],
                                         mybir.ActivationFunctionType.Exp,
                                         bias=nbias[:], scale=1.0)
                    # column-sum accumulation on DVE in bf16 (2x DVE rate,
                    # ~0.6us/add; the ~0.2% denominator rounding is well
                    # inside the error budget), emitted one tile behind the
                    # at-mults BEFORE tile mt's own mults so acc is complete
                    # ~1.2us earlier and the colsum matmuls mid-out-ct0
                    # never wait on the DVE; tile 7 folds into the colsum
                    # matmul.
                    if mt == 1:
                        nc.vector.tensor_copy(acc[:], at[:, 0])
                    elif mt > 1:
                        nc.vector.tensor_tensor(acc[:], acc[:], at[:, mt - 1],
                                                mybir.AluOpType.add)
                    # at[m, n] *= e12[m, n]  (plain packed bf16 mult, in
                    # place, full tile: one DVE op per tile keeps the
                    # near-saturated S-phase DVE chain (add + mult per
                    # 1.71us of PE work) under the PE pace)
                    nc.vector.tensor_tensor(at[:, mt], at[:, mt],
                                            e12s[mt][:],
                                            mybir.AluOpType.mult)

                # out = v A^T : [c-part, n], normalized by column sums.
                # The colsum matmuls (into pv-pool banks, idle after the v
                # phase) are emitted mid-way through out-ct0 so the DVE
                # reciprocal halves can overlap the remaining out matmuls;
                # each reciprocal half is interleaved between psO copies so
                # it never delays a pbig bank release.
                rrec = rrp.tile([P, N], F32, tag="rrec")
                obs = []
                pcss = []

                def norm_store(ct_):
                    # normalize in halves on GpSimd and DVE in parallel,
                    # casting to the bf16 store tile, DMAing each half as it
                    # completes.  Streamed from inside the out loop (obs[0]
                    # after ct2, obs[1] after ct3, rest behind the loop) so
                    # the stores overlap the remaining out matmuls and the
                    # end-of-kernel drain is ~2 half-size tiles instead of 4.
                    obf = obfp.tile([P, N], BF16, tag="obf")
                    for hb in range(2):
                        eng = nc.gpsimd if hb == 0 else nc.vector
                        sl = slice(hb * 512, (hb + 1) * 512)
                        eng.tensor_tensor(obf[:, sl], obs[ct_][:, sl],
                                          rrec[:, sl], mybir.AluOpType.mult)
                        nc.sync.dma_start(
                            o_ext[b, ct_ * P:(ct_ + 1) * P,
                                  hb * 512:(hb + 1) * 512],
                            obf[:, sl])

                for ct in range(CT):
                    if b == BLOC - 1 and ct >= 2:
                        # kernel-tail critical path: nb-major matmul order
                        # with a SEPARATE pvp PSUM tile per 512-col half
                        # (the pv banks idle after colsum/recips; separate
                        # tiles dodge the tile-granular WAR that serialized
                        # nb1's matmuls on nb0's eviction when both halves
                        # shared one psO tile).  Each half is normalized
                        # fused into the DVE eviction and stored the moment
                        # its 8-matmul sweep lands, alternating the issue
                        # between two DGE engines -- only the final 128KB
                        # half remains for the end-of-kernel drain.
                        if ct == 2:
                            norm_store(0)
                            norm_store(1)
                        obf = obfp.tile([P, N], BF16, tag="obf")
                        for nb in range(2):
                            sl = slice(nb * 512, (nb + 1) * 512)
                            psH = pvp.tile([P, 512], F32, tag="pv")
                            for mt in range(NT):
                                nc.tensor.matmul(
                                    psH[:],
                                    vt[:, mt, ct * P:(ct + 1) * P],
                                    at[:, mt, sl],
                                    start=(mt == 0), stop=(mt == NT - 1),
                                )
                            nc.vector.tensor_tensor(
                                obf[:, sl], psH[:], rrec[:, sl],
                                mybir.AluOpType.mult)
                            eng = nc.scalar if nb == 0 else nc.sync
                            eng.dma_start(
                                o_ext[b, ct * P:(ct + 1) * P, sl],
                                obf[:, sl])
                        continue
                    psO = pbig.tile([P, N], F32, tag="pbig")
                    for mt in range(NT):
                        for nb in range(2):
                            nc.tensor.matmul(
                                psO[:, nb * 512:(nb + 1) * 512],
                                vt[:, mt, ct * P:(ct + 1) * P],
                                at[:, mt, nb * 512:(nb + 1) * 512],
                                start=(mt == 0), stop=(mt == NT - 1),
                            )
                    if ct == 0:
                        # colsum broadcast, emitted AFTER ct0's matmuls:
                        # psCS[p, n] = sum_i acc[i, n] + sum_i at7[i, n].
                        # The at7/acc DVE chain finishes ~psT7+2us; at the
                        # post-loop emission point the PE arrives ~psT7+3.4us
                        # so neither the colsum nor out-mt7 ever waits.
                        for nb in range(2):
                            pcs = pvp.tile([P, 512], F32, tag="pv")
                            nc.tensor.matmul(
                                pcs[:],
                                ones_b[:],
                                acc[:, nb * 512:(nb + 1) * 512],
                                start=True, stop=False,
                            )
                            nc.tensor.matmul(
                                pcs[:],
                                ones_b[:],
                                at[:, NT - 1, nb * 512:(nb + 1) * 512],
                                start=False, stop=True,
                            )
                            pcss.append(pcs)
                    ob = osbp.tile([P, N], F32, tag="osb")
                    # psO -> SBUF copies on the Scalar engine (Copy shares
                    # the Exp activation table, so no table reload), keeping
                    # PSUM-bank releases off the DVE queue.
                    nc.scalar.activation(ob[:], psO[:],
                                         mybir.ActivationFunctionType.Copy,
                                         bias=0.0, scale=1.0)
                    if ct < 2:
                        # 18-bit ~0.7us approx reciprocal (vs 3.3us exact):
                        # colsums are strictly-positive well-normalized f32,
                        # far from the approx's undefined edge cases, and the
                        # denominator error budget is ~1e-3.
                        nc.vector.reciprocal_approx_fast(
                            rrec[:, ct * 512:(ct + 1) * 512], pcss[ct][:])
                    obs.append(ob)
                    if ct == 1 and b + 1 < BLOC:
                        xbf_next = xbfp.tile([P, CT, N], F16, tag="xbf")
                        nc.sync.dma_start(
                            xbf_next[:],
                            x_ext[b + 1].rearrange("(ct p) n -> p ct n", p=P))
                        Esb_next = emit_t(xbf_next)
                    if ct == 2:
                        norm_store(0)
                    if ct == 3:
                        norm_store(1)
                        norm_store(2)
                if b + 1 < BLOC:
                    # E^T burst for image b+1 sits between ct3's bf16
                    # matmuls and the f32r g phase: its two stationary-mode
                    # switches merge into the dtype seam that exists at the
                    # image boundary anyway, instead of splitting ct2/ct3.
                    esb_next = emit_trans(Esb_next)
                    norm_store(3)
                    xbf, Esb, esb = xbf_next, Esb_next, esb_next

    nc.compile()
    return nc


_NC_CACHE = None


def _get_nc():
    global _NC_CACHE
    if _NC_CACHE is None:
        _NC_CACHE = build_nc()
    return _NC_CACHE


def _prep_inputs(x, W, rel_h, rel_w):
    x = np.ascontiguousarray(np.asarray(x, dtype=np.float32))
    W = np.asarray(W, dtype=np.float32).astype(np.float64)
    rel_hm = np.asarray(rel_h, dtype=np.float32).reshape(C, H).astype(np.float64)
    rel_wm = np.asarray(rel_w, dtype=np.float32).reshape(C, W_).astype(np.float64)
    Wq, Wk, Wv = W[0:C], W[C:2 * C], W[2 * C:3 * C]
    # S = q^T k + pos^T q = x^T (Wq^T Wk) x + (Wq^T pos)^T x, with the rank-64
    # pos term separable into h- and w-factors applied post-exponentiation.
    mt_h = np.ascontiguousarray(
        (Wq.T @ Wk).T.astype(np.float32)).astype(np.float16)
    wvt_h = np.ascontiguousarray(
        Wv.T.astype(np.float32)).astype(np.float16)
    pfm = np.zeros((C, JW), np.float64)
    pfm[:, 0:H] = Wq.T @ rel_hm
    pfm[:, H:JW] = Wq.T @ rel_wm
    pf_h = pfm.astype(np.float32).astype(np.float16)
    xs = x.astype(np.float16).reshape(NCORES, BLOC, C, N)
    return xs, mt_h, wvt_h, pf_h


def _in_maps(inputs):
    xs, mt_h, wvt_h, pf_h = _prep_inputs(**inputs)
    return [
        {"x": np.ascontiguousarray(xs[i]), "MT": mt_h, "WVT": wvt_h,
         "PF": pf_h}
        for i in range(NCORES)
    ]


def kernel(x, W, rel_h, rel_w):
    nc = _get_nc()
    in_maps = _in_maps({"x": x, "W": W, "rel_h": rel_h, "rel_w": rel_w})
    res = run_bass_kernel_spmd(nc, in_maps, core_ids=list(range(NCORES)))
    out = np.concatenate(
        [np.asarray(res.results[i]["out"]).astype(np.float32)
         for i in range(NCORES)], axis=0)
    return out.reshape(B, C, H, W_)

